# revision 27
# baseline (speedup 1.0000x reference)
"""Multi-head attention (B=4, N=2048, D=768, H=12) on 8 trn2 NeuronCores.

Sharding: core c -> (batch b = c//2, head-half g = c%2).  Each core computes
the qkv projection for its 6 heads, attention, and a partial output
projection (over its 384 feature columns).  The host sums the two partials
per batch and adds the proj bias.  No collectives.

Device design (per core), all matmuls in bf16 (1 cyc/col stream, FWL
weight loads):
 - x is transposed on host to xT [768, 2048] so the contraction dim (c) is
   on SBUF partitions for both the Q/K (xT as rhs) and V (xT as lhsT)
   matmuls.
 - Q^T/K^T are produced as per-pair [128, 2048] bf16 tiles (head-dim on
   partitions; rows 0-63 = head 2p, 64-127 = head 2p+1), enabling row-tiled
   (K=64 x2) concurrent S^T matmuls.
 - S^T = K Q^T per (pair, k-tile, q-chunk); exp runs on ACT directly from
   PSUM with scale=1/8 folded in (no max subtraction: |scores*scale| < ~7);
   a fraction of k-tiles run a one-op Schraudolph exp on DVE instead
   (fp32->int16 tensor_scalar, bitcast to bf16) to keep ACT off the
   critical path.
 - V carries an appended ones-column so the AV matmul (out^T form, M=65)
   yields softmax denominators for free in row 64.
 - Normalization: reciprocal_approx_fast on DVE (single custom op), the
   fp32 result bitcast to f32r feeds a K=1 broadcast matmul, multiply on
   DVE; odd heads are DMA-moved to partitions 64..127 to build the proj
   lhsT layout.
 - One shared PSUM slot timeline (tag "s") for qkv/V/S^T/broadcast keeps all
   phases in one pool epoch so the scheduler can overlap them freely.
"""

import math

import ml_dtypes
import numpy as np

import concourse.bacc as bacc
import concourse.bass as bass  # noqa: F401
import concourse.mybir as mybir
import concourse.tile as tile
from concourse.bass_utils import run_bass_kernel_spmd

P = 128
NQ = 2048          # sequence length
CD = 768           # model dim
NHC = 6            # heads per core
DH = 64            # head dim
SCALE = DH ** -0.5
CT = CD // P       # 6 c-tiles
KT = NQ // P       # 16 k-tiles
QC = 512           # q chunk
NQC = NQ // QC     # 4
PAIRS = NHC // 2   # 3

F32 = mybir.dt.float32
F32R = mybir.dt.float32r
BF16 = mybir.dt.bfloat16
I16 = mybir.dt.int16

EXP_BIAS = -4.1588830833596715   # -ln(64); keeps denominators small
# Schraudolph constants for bf16 output: bits = round(x*2^7/ln2 + B)
SCH_A = 2.0 ** 7 / math.log(2.0)
SCH_B = 127.0 * 2 ** 7 - 7.4
# which k-tiles use the DVE fast-exp instead of ACT, per q-chunk (the
# attention inner loop is ACT-bound; early chunks overlap qkv PE work so
# they offload less, the final chunk has no PE slack so it offloads more;
# total error measured ~1.4e-2 < 2e-2)
DVE_EXP_KTS = {
    0: frozenset((5, 11)),
    1: frozenset((3, 7, 11, 15)),
    2: frozenset((3, 7, 11, 15)),
    3: frozenset((2, 5, 8, 10, 13, 15)),
}


def build_nc(n_reps=1):
    nc = bacc.Bacc("TRN2", debug=False, num_devices=8)

    xT_d = nc.dram_tensor("xT", [CD, NQ], BF16, kind="ExternalInput")
    wqkvT_d = nc.dram_tensor("wqkvT", [CD, 3 * 384], BF16, kind="ExternalInput")
    bqk_d = nc.dram_tensor("b_qk", [P, 6], F32, kind="ExternalInput")
    bv_d = nc.dram_tensor("b_v", [1, 384], BF16, kind="ExternalInput")
    wpT_d = nc.dram_tensor("wpT", [384, CD], BF16, kind="ExternalInput")
    onesb_d = nc.dram_tensor("onesb", [P, P], BF16, kind="ExternalInput")
    out_d = nc.dram_tensor("out", [NQ, CD], F32, kind="ExternalOutput")

    with tile.TileContext(nc) as tc:
        with (
            tc.tile_pool(name="consts", bufs=1) as consts,
            tc.tile_pool(name="big", bufs=1) as big,
            tc.tile_pool(name="attn", bufs=2) as attn_pool,
            tc.tile_pool(name="aT", bufs=6) as aT_pool,
            tc.tile_pool(name="norm", bufs=1) as norm_pool,
            tc.tile_pool(name="outst", bufs=2) as outst_pool,
            tc.tile_pool(name="ps_s", bufs=2, space="PSUM") as ps_s,
            tc.tile_pool(name="ps_av", bufs=1, space="PSUM") as ps_av,
            tc.tile_pool(name="ps_proj", bufs=1, space="PSUM") as ps_proj,
        ):
            # ---- constants (per-c-tile tiles so compute starts ASAP) ----
            xT_sb = [consts.tile([P, NQ], BF16, tag=f"xT{ct}", name=f"xT{ct}")
                     for ct in range(CT)]
            wq_sb = [consts.tile([P, 3 * 384], BF16, tag=f"wqkvT{ct}",
                                 name=f"wqkvT{ct}") for ct in range(CT)]
            # startup loads alternate between the two HWDGE engines (SP and
            # ACT have independent hardware DMA queues) so the first S^T
            # inputs land in half the time; issue order = first-use order.
            def eng(i):
                return nc.sync if i % 2 == 0 else nc.scalar

            def load_w_piece(piece):
                for ct in range(CT):
                    eng(ct).dma_start(
                        wq_sb[ct][:, piece * 384:(piece + 1) * 384],
                        wqkvT_d[ct * P:(ct + 1) * P, piece * 384:(piece + 1) * 384])

            # tiny consts first (1KB total): the first bias-adds and the V
            # bias matmul need them, and queued behind 6MB of bulk they'd
            # land last
            bqk_sb = consts.tile([P, 6], F32, tag="bqk")
            nc.sync.dma_start(bqk_sb[:, :], bqk_d[:, :])
            bv_sb = consts.tile([1, 384], BF16, tag="bv")
            nc.scalar.dma_start(bv_sb[:, :], bv_d[:, :])
            onesb_sb = consts.tile([P, P], BF16, tag="onesb")
            nc.sync.dma_start(onesb_sb[:, :], onesb_d[:, :])
            load_w_piece(1)
            for ct in range(CT):   # xT chunk 0 (feeds all of pair 0's S^T)
                eng(ct).dma_start(
                    xT_sb[ct][:, 0:QC], xT_d[ct * P:(ct + 1) * P, 0:QC])
            load_w_piece(2)   # V weights: the early v-tiles fill the PE
            load_w_piece(0)   # while the xT tail streams in
            for ct in range(CT):   # xT tail, one big DMA per c-tile
                eng(ct).dma_start(
                    xT_sb[ct][:, QC:NQ], xT_d[ct * P:(ct + 1) * P, QC:NQ])
            wp_sb = []
            for t3 in range(3):
                w = consts.tile([P, CD], BF16, tag=f"wpT{t3}")
                nc.sync.dma_start(w[:, :], wpT_d[t3 * P:(t3 + 1) * P, :])
                wp_sb.append(w)
            expb_sb = consts.tile([P, 1], F32, tag="expb")
            nc.vector.memset(expb_sb[:, :], EXP_BIAS)

            for _rep in range(n_reps):
                # ---- persistent activations ----
                # per-pair Q^T/K^T [128, 2048]: rows 0-63 head 2p, 64-127 head 2p+1
                q_sb = [big.tile([P, NQ], BF16, tag=f"q{p}", name=f"q{p}") for p in range(PAIRS)]
                k_sb = [big.tile([P, NQ], BF16, tag=f"k{p}", name=f"k{p}") for p in range(PAIRS)]
                # v[part=k-position, k-tile, head, 66]; col 64 = ones
                v_sb = big.tile([P, KT, NHC, DH + 2], BF16, tag="v")
                nc.sync.dma_start(
                    v_sb[:, :, :, DH],
                    onesb_d[:, 0:KT * NHC].rearrange("p (a b) -> p a b", a=KT),
                )

                def qk_unit(kind, t, qc):
                    # Q^T (kind 0) / K^T (kind 1) pair-tile t, one 512-chunk
                    dest = (q_sb if kind == 0 else k_sb)[t]
                    col0 = kind * 384 + t * P
                    ps = ps_s.tile([P, 2, QC], F32, tag="s")
                    for ct in range(CT):
                        nc.tensor.matmul(
                            ps[:, 0, :],
                            lhsT=wq_sb[ct][:, col0:col0 + P],
                            rhs=xT_sb[ct][:, qc * QC:(qc + 1) * QC],
                            start=(ct == 0),
                            stop=(ct == CT - 1),
                        )
                    nc.vector.tensor_scalar_add(
                        out=dest[:, qc * QC:(qc + 1) * QC],
                        in0=ps[:, 0, :],
                        scalar1=bqk_sb[:, kind * 3 + t:kind * 3 + t + 1],
                    )

                def v_tile(nt):
                    ps = ps_s.tile([P, 2, QC], F32, tag="s")
                    for ct in range(CT):
                        nc.tensor.matmul(
                            ps[:, 0, 0:384],
                            lhsT=xT_sb[ct][:, nt * P:(nt + 1) * P],
                            rhs=wq_sb[ct][:, 768:1152],
                            start=(ct == 0),
                            stop=False,
                        )
                    # bias via K=1 ones-row matmul
                    nc.tensor.matmul(
                        ps[:, 0, 0:384],
                        lhsT=onesb_sb[0:1, :],
                        rhs=bv_sb[0:1, :],
                        start=False,
                        stop=True,
                    )
                    nc.vector.tensor_copy(
                        out=v_sb[:, nt, :, 0:DH],
                        in_=ps[:, 0, 0:384].rearrange("p (h d) -> p h d", h=NHC),
                    )

                at_chunks = {}

                def attn_pair(qc, pr, emit_v_from=KT, pending=None,
                              tail=False):
                    qsl = slice(qc * QC, (qc + 1) * QC)
                    if pr == 0:
                        at_chunks[qc] = attn_pool.tile([P, PAIRS, QC], BF16,
                                                       tag="attnT", name="at_chunk")
                    at_chunk = at_chunks[qc]
                    av = ps_av.tile([DH + 1, 2, QC], F32, tag="av")
                    LOOKAHEAD = 2   # S^T/exp run ahead of AV so the PE queue
                    a_ts = {}       # has work while the av slot drains

                    def st_exp(kt):
                        sp = ps_s.tile([P, 2, QC], F32, tag="s")
                        for h2 in range(2):
                            nc.tensor.matmul(
                                sp[:, h2, :],
                                lhsT=k_sb[pr][h2 * DH:(h2 + 1) * DH,
                                              kt * P:(kt + 1) * P],
                                rhs=q_sb[pr][h2 * DH:(h2 + 1) * DH, qsl],
                                start=True,
                                stop=True,
                                tile_position=(h2 * DH, 0),
                            )
                        a_t = aT_pool.tile([P, 2, QC], BF16, tag="aT")
                        if kt >= emit_v_from:
                            # after S^T so the PE queue never stalls on the
                            # V-weight DMAs while S^T work is ready
                            v_tile(kt)
                        if kt in DVE_EXP_KTS[qc]:
                            # Schraudolph: bf16 bits = round(s*SCALE*A + B)
                            nc.vector.tensor_scalar(
                                out=a_t[:, :, :].bitcast(I16),
                                in0=sp[:, :, :],
                                scalar1=float(SCH_A * SCALE),
                                scalar2=float(SCH_B + SCH_A * EXP_BIAS),
                                op0=mybir.AluOpType.mult,
                                op1=mybir.AluOpType.add,
                            )
                        else:
                            nc.scalar.activation(
                                out=a_t[:, :, :],
                                in_=sp[:, :, :],
                                func=mybir.ActivationFunctionType.Exp,
                                bias=expb_sb[:, 0:1],
                                scale=float(SCALE),
                            )
                        a_ts[kt] = a_t

                    def av_mm(kt):
                        a_t = a_ts.pop(kt)
                        for h2 in range(2):
                            nc.tensor.matmul(
                                av[:, h2, :],
                                lhsT=v_sb[:, kt, pr * 2 + h2, 0:DH + 1],
                                rhs=a_t[:, h2, :],
                                start=(kt == 0),
                                stop=(kt == KT - 1),
                            )

                    # 2-k-tile batches: [S^T,S^T] then [AV,AV] so the PE
                    # switches tiling mode (row-tiled S^T vs full-array AV)
                    # half as often -- each mode change drains the array.
                    for ktp in range(0, KT, 2):
                        st_exp(ktp)
                        st_exp(ktp + 1)
                        if ktp == 4 and pending is not None:
                            pending()   # previous pair's deferred normalization
                        if ktp >= LOOKAHEAD:
                            av_mm(ktp - LOOKAHEAD)
                            av_mm(ktp - LOOKAHEAD + 1)
                    for kt in range(KT - LOOKAHEAD, KT):
                        av_mm(kt)
                    # evacuate av psum early (frees the slot for the next
                    # pair); the normalization itself is deferred into the
                    # next pair's kt loop so the reciprocal chain never gates
                    # the PE queue
                    avc = norm_pool.tile([DH + 1, 2, QC], F32, tag="avc")
                    nc.vector.tensor_copy(out=avc[:, :, :], in_=av[:, :, :])

                    def norm():
                        rcf = norm_pool.tile([DH + 1, 2, QC], F32, tag="rcf")
                        # NB: reciprocal_approx_fast miscomputes on 1-partition
                        # APs; run it over all 65 rows (partition-parallel,
                        # same cost) and consume only the denominator row.
                        nc.vector.reciprocal_approx_fast(
                            out=rcf[:, :, :],
                            in_=avc[:, :, :],
                        )
                        # gpsimd-initiated DMA casts fp32->bf16 off the
                        # hot engines (only SWDGE DMAs can cast)
                        rc = norm_pool.tile([DH + 1, 2, QC], BF16, tag="rc")
                        if tail:
                            # last pair of the kernel: the norm chain is
                            # exposed on the tail, so cast on DVE instead of
                            # a ~2us gpsimd-initiated DMA
                            nc.vector.tensor_copy(out=rc[DH:DH + 1, :, :],
                                                  in_=rcf[DH:DH + 1, :, :])
                        else:
                            nc.gpsimd.dma_start(rc[DH:DH + 1, :, :],
                                                rcf[DH:DH + 1, :, :])
                        for h2 in range(2):
                            bc_ps = ps_proj.tile([P, 2, QC], F32, tag="proj")
                            nc.tensor.matmul(
                                bc_ps[:, h2, :],
                                lhsT=onesb_sb[DH:DH + 1, :],
                                rhs=rc[DH:DH + 1, h2, :],
                                start=True,
                                stop=True,
                            )
                            if h2 == 0:
                                nc.vector.tensor_mul(
                                    out=at_chunk[0:DH, pr, :],
                                    in0=avc[0:DH, 0, :],
                                    in1=bc_ps[0:DH, h2, :],
                                )
                            else:
                                tmp = norm_pool.tile([DH, QC], BF16, tag="tmp1")
                                nc.vector.tensor_mul(
                                    out=tmp[:, :],
                                    in0=avc[0:DH, 1, :],
                                    in1=bc_ps[0:DH, h2, :],
                                )
                                nc.sync.dma_start(at_chunk[DH:P, pr, :],
                                                  tmp[:, :])
                    return norm

                def proj_chunk(qc):
                    at_chunk = at_chunks[qc]
                    last = qc == NQC - 1
                    for sub in range(QC // P):
                        # final chunk only: alternate into the (drained) av
                        # banks so consecutive subs double-buffer -- its proj
                        # pipeline is fully exposed on the kernel tail, and
                        # with no next-chunk attention the av slot has no
                        # other consumer to couple with
                        if last and sub % 2 == 1:
                            pp = ps_av.tile([P, 2, QC], F32, tag="av")
                        else:
                            pp = ps_proj.tile([P, 2, QC], F32, tag="proj")
                        for t3 in range(PAIRS):
                            for (bank, o0, ow) in ((0, 0, 512), (1, 512, 256)):
                                nc.tensor.matmul(
                                    pp[:, bank, 0:ow],
                                    lhsT=at_chunk[:, t3, sub * P:(sub + 1) * P],
                                    rhs=wp_sb[t3][:, o0:o0 + ow],
                                    start=(t3 == 0),
                                    stop=(t3 == PAIRS - 1),
                                )
                        ost = outst_pool.tile([P, CD], F32, tag="ost")
                        nc.vector.tensor_copy(out=ost[:, 0:512], in_=pp[:, 0, :])
                        nc.vector.tensor_copy(out=ost[:, 512:CD], in_=pp[:, 1, 0:256])
                        n0 = qc * QC + sub * P
                        nc.sync.dma_start(out_d[n0:n0 + P, :], ost[:, :])

                def qk_tile(kind, t):
                    for qc in range(NQC):
                        qk_unit(kind, t, qc)

                # software-pipelined emission: K pair0 + Q pair0 chunk0 are
                # all that chunk-0/pair-0 attention needs, so emit them first
                # (with V production interleaved per k-tile) to start ACT as
                # early as the xT DMA allows.
                qk_unit(1, 0, 0)
                qk_unit(0, 0, 0)
                # v-tiles 0-7 need only xT chunks 0-1 + the V weights: they
                # fill the PE during the xT-tail DMA window that the later
                # K^T chunks head-block on
                for nt in range(8):
                    v_tile(nt)
                for c in range(1, NQC):
                    qk_unit(1, 0, c)
                    qk_unit(0, 0, c)
                    # v-tile nt needs xT chunk nt//4; tiles 0-7 went above
                    for nt in range(4 * c, 4 * c + 4):
                        if nt >= 8:
                            v_tile(nt)
                pending = attn_pair(0, 0, emit_v_from=KT)
                for pr in range(1, PAIRS):
                    qk_tile(0, pr)
                    qk_tile(1, pr)
                    pending = attn_pair(0, pr, pending=pending)
                # the deferred norm flows across chunk boundaries: chunk
                # qc's last norm runs inside chunk qc+1's first pair, and
                # proj(qc) is emitted after that pair so its matmuls (which
                # wait on the norm chain) never head-block the next chunk's
                # S^T stream in the PE queue
                for qc in range(1, NQC):
                    for pr in range(PAIRS):
                        pending = attn_pair(
                            qc, pr, pending=pending,
                            tail=(qc == NQC - 1 and pr == PAIRS - 1))
                        if pr == 0:
                            proj_chunk(qc - 1)
                pending()
                pending = None
                proj_chunk(NQC - 1)

    nc.finalize()
    return nc


_NC = None


def _get_nc():
    global _NC
    if _NC is None:
        _NC = build_nc()
    return _NC


def _make_in_maps(inputs):
    x = np.asarray(inputs["x"], dtype=np.float32)
    w_qkv = np.asarray(inputs["w_qkv"], dtype=np.float32)
    b_qkv = np.asarray(inputs["b_qkv"], dtype=np.float32)
    w_proj = np.asarray(inputs["w_proj"], dtype=np.float32)

    bf = ml_dtypes.bfloat16
    in_maps = []
    for c in range(8):
        b, g = c // 2, c % 2
        sl = slice(384 * g, 384 * g + 384)
        xT = np.ascontiguousarray(x[b].T)                       # [768, 2048]
        wq = w_qkv[0:768][sl]                                    # [384, 768]
        wk = w_qkv[768:1536][sl]
        wv = w_qkv[1536:2304][sl]
        wqkvT = np.ascontiguousarray(np.concatenate([wq, wk, wv], axis=0).T)
        bq = b_qkv[0:768][sl]
        bk = b_qkv[768:1536][sl]
        bv = b_qkv[1536:2304][sl]
        b_qk = np.ascontiguousarray(
            np.concatenate([bq, bk]).reshape(6, P).T)            # [128, 6]
        wpT = np.ascontiguousarray(w_proj[:, sl].T)
        in_maps.append({
            "onesb": np.ones((P, P), dtype=bf),
            "xT": xT.astype(bf),
            "wqkvT": wqkvT.astype(bf),
            "b_qk": b_qk,
            "b_v": np.ascontiguousarray(bv.reshape(1, 384)).astype(bf),
            "wpT": wpT.astype(bf),
        })
    return in_maps


def _run(inputs, trace=False):
    nc = _get_nc()
    in_maps = _make_in_maps(inputs)
    res = run_bass_kernel_spmd(nc, in_maps, core_ids=list(range(8)), trace=trace)
    b_proj = np.asarray(inputs["b_proj"], dtype=np.float32)
    out = np.empty((4, NQ, CD), dtype=np.float32)
    for b in range(4):
        out[b] = res.results[2 * b]["out"] + res.results[2 * b + 1]["out"] + b_proj
    return out, res


def kernel(**inputs) -> np.ndarray:
    out, _ = _run(inputs, trace=False)
    return out



# revision 28
# speedup vs baseline: 1.0163x; 1.0163x over previous
"""Multi-head attention (B=4, N=2048, D=768, H=12) on 8 trn2 NeuronCores.

Sharding: core c -> (batch b = c//2, head-half g = c%2).  Each core computes
the qkv projection for its 6 heads, attention, and a partial output
projection (over its 384 feature columns).  The host sums the two partials
per batch and adds the proj bias.  No collectives.

Device design (per core), all matmuls in bf16 (1 cyc/col stream, FWL
weight loads):
 - x is transposed on host to xT [768, 2048] so the contraction dim (c) is
   on SBUF partitions for both the Q/K (xT as rhs) and V (xT as lhsT)
   matmuls.
 - Q^T/K^T are produced as per-pair [128, 2048] bf16 tiles (head-dim on
   partitions; rows 0-63 = head 2p, 64-127 = head 2p+1), enabling row-tiled
   (K=64 x2) concurrent S^T matmuls.
 - S^T = K Q^T per (pair, k-tile, q-chunk); exp runs on ACT directly from
   PSUM with scale=1/8 folded in (no max subtraction: |scores*scale| < ~7);
   a fraction of k-tiles run a one-op Schraudolph exp on DVE instead
   (fp32->int16 tensor_scalar, bitcast to bf16) to keep ACT off the
   critical path.
 - V carries an appended ones-column so the AV matmul (out^T form, M=65)
   yields softmax denominators for free in row 64.
 - Normalization: reciprocal_approx_fast on DVE (single custom op), the
   fp32 result bitcast to f32r feeds a K=1 broadcast matmul, multiply on
   DVE; odd heads are DMA-moved to partitions 64..127 to build the proj
   lhsT layout.
 - One shared PSUM slot timeline (tag "s") for qkv/V/S^T/broadcast keeps all
   phases in one pool epoch so the scheduler can overlap them freely.
"""

import math

import ml_dtypes
import numpy as np

import concourse.bacc as bacc
import concourse.bass as bass  # noqa: F401
import concourse.mybir as mybir
import concourse.tile as tile
from concourse.bass_utils import run_bass_kernel_spmd

P = 128
NQ = 2048          # sequence length
CD = 768           # model dim
NHC = 6            # heads per core
DH = 64            # head dim
SCALE = DH ** -0.5
CT = CD // P       # 6 c-tiles
KT = NQ // P       # 16 k-tiles
QC = 512           # q chunk
NQC = NQ // QC     # 4
PAIRS = NHC // 2   # 3

F32 = mybir.dt.float32
F32R = mybir.dt.float32r
BF16 = mybir.dt.bfloat16
I16 = mybir.dt.int16

EXP_BIAS = -4.1588830833596715   # -ln(64); keeps denominators small
# Schraudolph constants for bf16 output: bits = round(x*2^7/ln2 + B)
SCH_A = 2.0 ** 7 / math.log(2.0)
SCH_B = 127.0 * 2 ** 7 - 7.4
# which k-tiles use the DVE fast-exp instead of ACT, per q-chunk (the
# attention inner loop is ACT-bound; early chunks overlap qkv PE work so
# they offload less, the final chunk has no PE slack so it offloads more;
# total error measured ~1.4e-2 < 2e-2)
DVE_EXP_KTS = {
    0: frozenset((5, 11)),
    1: frozenset((3, 7, 11, 15)),
    2: frozenset((3, 7, 11, 15)),
    3: frozenset((2, 5, 8, 10, 13, 15)),
}


def build_nc(n_reps=1):
    nc = bacc.Bacc("TRN2", debug=False, num_devices=8)

    xT_d = nc.dram_tensor("xT", [CD, NQ], BF16, kind="ExternalInput")
    wqkvT_d = nc.dram_tensor("wqkvT", [CD, 3 * 384], BF16, kind="ExternalInput")
    bqk_d = nc.dram_tensor("b_qk", [P, 6], F32, kind="ExternalInput")
    bv_d = nc.dram_tensor("b_v", [1, 384], BF16, kind="ExternalInput")
    wpT_d = nc.dram_tensor("wpT", [384, CD], BF16, kind="ExternalInput")
    onesb_d = nc.dram_tensor("onesb", [P, P], BF16, kind="ExternalInput")
    out_d = nc.dram_tensor("out", [NQ, CD], F32, kind="ExternalOutput")

    with tile.TileContext(nc) as tc:
        with (
            tc.tile_pool(name="consts", bufs=1) as consts,
            tc.tile_pool(name="big", bufs=1) as big,
            tc.tile_pool(name="attn", bufs=2) as attn_pool,
            tc.tile_pool(name="aT", bufs=6) as aT_pool,
            tc.tile_pool(name="norm", bufs=1) as norm_pool,
            tc.tile_pool(name="outst", bufs=2) as outst_pool,
            tc.tile_pool(name="ps_s", bufs=2, space="PSUM") as ps_s,
            tc.tile_pool(name="ps_av", bufs=1, space="PSUM") as ps_av,
            tc.tile_pool(name="ps_proj", bufs=1, space="PSUM") as ps_proj,
        ):
            # ---- constants (per-c-tile tiles so compute starts ASAP) ----
            xT_sb = [consts.tile([P, NQ], BF16, tag=f"xT{ct}", name=f"xT{ct}")
                     for ct in range(CT)]
            wq_sb = [consts.tile([P, 3 * 384], BF16, tag=f"wqkvT{ct}",
                                 name=f"wqkvT{ct}") for ct in range(CT)]
            # startup loads alternate between the two HWDGE engines (SP and
            # ACT have independent hardware DMA queues) so the first S^T
            # inputs land in half the time; issue order = first-use order.
            def eng(i):
                return nc.sync if i % 2 == 0 else nc.scalar

            def load_w_piece(piece):
                for ct in range(CT):
                    eng(ct).dma_start(
                        wq_sb[ct][:, piece * 384:(piece + 1) * 384],
                        wqkvT_d[ct * P:(ct + 1) * P, piece * 384:(piece + 1) * 384])

            # tiny consts first (1KB total): the first bias-adds and the V
            # bias matmul need them, and queued behind 6MB of bulk they'd
            # land last
            bqk_sb = consts.tile([P, 6], F32, tag="bqk")
            nc.sync.dma_start(bqk_sb[:, :], bqk_d[:, :])
            bv_sb = consts.tile([1, 384], BF16, tag="bv")
            nc.scalar.dma_start(bv_sb[:, :], bv_d[:, :])
            onesb_sb = consts.tile([P, P], BF16, tag="onesb")
            nc.sync.dma_start(onesb_sb[:, :], onesb_d[:, :])
            load_w_piece(1)
            for ct in range(CT):   # xT chunk 0 (feeds all of pair 0's S^T)
                eng(ct).dma_start(
                    xT_sb[ct][:, 0:QC], xT_d[ct * P:(ct + 1) * P, 0:QC])
            load_w_piece(2)   # V weights: the early v-tiles fill the PE
            load_w_piece(0)   # while the xT tail streams in
            for ct in range(CT):   # xT tail, one big DMA per c-tile
                eng(ct).dma_start(
                    xT_sb[ct][:, QC:NQ], xT_d[ct * P:(ct + 1) * P, QC:NQ])
            wp_sb = []
            for t3 in range(3):
                w = consts.tile([P, CD], BF16, tag=f"wpT{t3}")
                nc.sync.dma_start(w[:, :], wpT_d[t3 * P:(t3 + 1) * P, :])
                wp_sb.append(w)
            expb_sb = consts.tile([P, 1], F32, tag="expb")
            nc.vector.memset(expb_sb[:, :], EXP_BIAS)

            for _rep in range(n_reps):
                # ---- persistent activations ----
                # per-pair Q^T/K^T [128, 2048]: rows 0-63 head 2p, 64-127 head 2p+1
                q_sb = [big.tile([P, NQ], BF16, tag=f"q{p}", name=f"q{p}") for p in range(PAIRS)]
                k_sb = [big.tile([P, NQ], BF16, tag=f"k{p}", name=f"k{p}") for p in range(PAIRS)]
                # v[part=k-position, k-tile, head, 66]; col 64 = ones
                v_sb = big.tile([P, KT, NHC, DH + 2], BF16, tag="v")
                nc.sync.dma_start(
                    v_sb[:, :, :, DH],
                    onesb_d[:, 0:KT * NHC].rearrange("p (a b) -> p a b", a=KT),
                )

                def qk_unit(kind, t, qc):
                    # Q^T (kind 0) / K^T (kind 1) pair-tile t, one 512-chunk
                    dest = (q_sb if kind == 0 else k_sb)[t]
                    col0 = kind * 384 + t * P
                    ps = ps_s.tile([P, 2, QC], F32, tag="s")
                    for ct in range(CT):
                        nc.tensor.matmul(
                            ps[:, 0, :],
                            lhsT=wq_sb[ct][:, col0:col0 + P],
                            rhs=xT_sb[ct][:, qc * QC:(qc + 1) * QC],
                            start=(ct == 0),
                            stop=(ct == CT - 1),
                        )
                    nc.vector.tensor_scalar_add(
                        out=dest[:, qc * QC:(qc + 1) * QC],
                        in0=ps[:, 0, :],
                        scalar1=bqk_sb[:, kind * 3 + t:kind * 3 + t + 1],
                    )

                def v_tile(nt):
                    ps = ps_s.tile([P, 2, QC], F32, tag="s")
                    for ct in range(CT):
                        nc.tensor.matmul(
                            ps[:, 0, 0:384],
                            lhsT=xT_sb[ct][:, nt * P:(nt + 1) * P],
                            rhs=wq_sb[ct][:, 768:1152],
                            start=(ct == 0),
                            stop=False,
                        )
                    # bias via K=1 ones-row matmul
                    nc.tensor.matmul(
                        ps[:, 0, 0:384],
                        lhsT=onesb_sb[0:1, :],
                        rhs=bv_sb[0:1, :],
                        start=False,
                        stop=True,
                    )
                    nc.vector.tensor_copy(
                        out=v_sb[:, nt, :, 0:DH],
                        in_=ps[:, 0, 0:384].rearrange("p (h d) -> p h d", h=NHC),
                    )

                at_chunks = {}

                def attn_pair(qc, pr, emit_v_from=KT, pending=None,
                              tail=False):
                    qsl = slice(qc * QC, (qc + 1) * QC)
                    if pr == 0:
                        at_chunks[qc] = attn_pool.tile([P, PAIRS, QC], BF16,
                                                       tag="attnT", name="at_chunk")
                    at_chunk = at_chunks[qc]
                    av = ps_av.tile([DH + 1, 2, QC], F32, tag="av")
                    LOOKAHEAD = 2   # S^T/exp run ahead of AV so the PE queue
                    a_ts = {}       # has work while the av slot drains

                    def st_exp(kt):
                        sp = ps_s.tile([P, 2, QC], F32, tag="s")
                        for h2 in range(2):
                            nc.tensor.matmul(
                                sp[:, h2, :],
                                lhsT=k_sb[pr][h2 * DH:(h2 + 1) * DH,
                                              kt * P:(kt + 1) * P],
                                rhs=q_sb[pr][h2 * DH:(h2 + 1) * DH, qsl],
                                start=True,
                                stop=True,
                                tile_position=(h2 * DH, 0),
                            )
                        a_t = aT_pool.tile([P, 2, QC], BF16, tag="aT")
                        if kt >= emit_v_from:
                            # after S^T so the PE queue never stalls on the
                            # V-weight DMAs while S^T work is ready
                            v_tile(kt)
                        if kt in DVE_EXP_KTS[qc]:
                            # Schraudolph: bf16 bits = round(s*SCALE*A + B)
                            nc.vector.tensor_scalar(
                                out=a_t[:, :, :].bitcast(I16),
                                in0=sp[:, :, :],
                                scalar1=float(SCH_A * SCALE),
                                scalar2=float(SCH_B + SCH_A * EXP_BIAS),
                                op0=mybir.AluOpType.mult,
                                op1=mybir.AluOpType.add,
                            )
                        else:
                            nc.scalar.activation(
                                out=a_t[:, :, :],
                                in_=sp[:, :, :],
                                func=mybir.ActivationFunctionType.Exp,
                                bias=expb_sb[:, 0:1],
                                scale=float(SCALE),
                            )
                        a_ts[kt] = a_t

                    def av_mm(kt):
                        a_t = a_ts.pop(kt)
                        for h2 in range(2):
                            nc.tensor.matmul(
                                av[:, h2, :],
                                lhsT=v_sb[:, kt, pr * 2 + h2, 0:DH + 1],
                                rhs=a_t[:, h2, :],
                                start=(kt == 0),
                                stop=(kt == KT - 1),
                            )

                    # 2-k-tile batches: [S^T,S^T] then [AV,AV] so the PE
                    # switches tiling mode (row-tiled S^T vs full-array AV)
                    # half as often -- each mode change drains the array.
                    for ktp in range(0, KT, 2):
                        st_exp(ktp)
                        st_exp(ktp + 1)
                        if ktp == 4 and pending is not None:
                            pending()   # previous pair's deferred normalization
                        if ktp >= LOOKAHEAD:
                            av_mm(ktp - LOOKAHEAD)
                            av_mm(ktp - LOOKAHEAD + 1)
                    for kt in range(KT - LOOKAHEAD, KT):
                        av_mm(kt)
                    # evacuate av psum early (frees the slot for the next
                    # pair); the normalization itself is deferred into the
                    # next pair's kt loop so the reciprocal chain never gates
                    # the PE queue
                    avc = norm_pool.tile([DH + 1, 2, QC], F32, tag="avc")
                    nc.vector.tensor_copy(out=avc[:, :, :], in_=av[:, :, :])

                    def norm():
                        rcf = norm_pool.tile([DH + 1, 2, QC], F32, tag="rcf")
                        # NB: reciprocal_approx_fast miscomputes on 1-partition
                        # APs; run it over all 65 rows (partition-parallel,
                        # same cost) and consume only the denominator row.
                        nc.vector.reciprocal_approx_fast(
                            out=rcf[:, :, :],
                            in_=avc[:, :, :],
                        )
                        # gpsimd-initiated DMA casts fp32->bf16 off the
                        # hot engines (only SWDGE DMAs can cast)
                        rc = norm_pool.tile([DH + 1, 2, QC], BF16, tag="rc")
                        if tail:
                            # last pair of the kernel: the norm chain is
                            # exposed on the tail, so cast on DVE instead of
                            # a ~2us gpsimd-initiated DMA
                            nc.vector.tensor_copy(out=rc[DH:DH + 1, :, :],
                                                  in_=rcf[DH:DH + 1, :, :])
                        else:
                            nc.gpsimd.dma_start(rc[DH:DH + 1, :, :],
                                                rcf[DH:DH + 1, :, :])
                        for h2 in range(2):
                            bc_ps = ps_proj.tile([P, 2, QC], F32, tag="proj")
                            nc.tensor.matmul(
                                bc_ps[:, h2, :],
                                lhsT=onesb_sb[DH:DH + 1, :],
                                rhs=rc[DH:DH + 1, h2, :],
                                start=True,
                                stop=True,
                            )
                            if h2 == 0:
                                nc.vector.tensor_mul(
                                    out=at_chunk[0:DH, pr, :],
                                    in0=avc[0:DH, 0, :],
                                    in1=bc_ps[0:DH, h2, :],
                                )
                            else:
                                tmp = norm_pool.tile([DH, QC], BF16, tag="tmp1")
                                nc.vector.tensor_mul(
                                    out=tmp[:, :],
                                    in0=avc[0:DH, 1, :],
                                    in1=bc_ps[0:DH, h2, :],
                                )
                                nc.sync.dma_start(at_chunk[DH:P, pr, :],
                                                  tmp[:, :])
                    return norm

                def proj_chunk(qc):
                    at_chunk = at_chunks[qc]
                    last = qc == NQC - 1
                    for sub in range(QC // P):
                        # final chunk only: alternate into the (drained) av
                        # banks so consecutive subs double-buffer -- its proj
                        # pipeline is fully exposed on the kernel tail, and
                        # with no next-chunk attention the av slot has no
                        # other consumer to couple with
                        if last and sub % 2 == 1:
                            pp = ps_av.tile([P, 2, QC], F32, tag="av")
                        else:
                            pp = ps_proj.tile([P, 2, QC], F32, tag="proj")
                        for t3 in range(PAIRS):
                            for (bank, o0, ow) in ((0, 0, 512), (1, 512, 256)):
                                nc.tensor.matmul(
                                    pp[:, bank, 0:ow],
                                    lhsT=at_chunk[:, t3, sub * P:(sub + 1) * P],
                                    rhs=wp_sb[t3][:, o0:o0 + ow],
                                    start=(t3 == 0),
                                    stop=(t3 == PAIRS - 1),
                                )
                        ost = outst_pool.tile([P, CD], F32, tag="ost")
                        nc.vector.tensor_copy(out=ost[:, 0:512], in_=pp[:, 0, :])
                        nc.vector.tensor_copy(out=ost[:, 512:CD], in_=pp[:, 1, 0:256])
                        n0 = qc * QC + sub * P
                        nc.sync.dma_start(out_d[n0:n0 + P, :], ost[:, :])

                def qk_tile(kind, t):
                    for qc in range(NQC):
                        qk_unit(kind, t, qc)

                # software-pipelined emission: K pair0 + Q pair0 chunk0 are
                # all that chunk-0/pair-0 attention needs, so emit them first
                # (with V production interleaved per k-tile) to start ACT as
                # early as the xT DMA allows.
                # prefetch order = DMA arrival order, so no unit head-blocks
                # the PE queue on a transfer a later-emitted unit doesn't
                # need: K0 (piece1+xTc0), v0-3 (piece2), Q0 (piece0), then
                # the tail-gated chunk units interleaved with their v-tiles
                qk_unit(1, 0, 0)
                for nt in range(4):
                    v_tile(nt)
                qk_unit(0, 0, 0)
                for c in range(1, NQC):
                    for nt in range(4 * c, 4 * c + 4):
                        v_tile(nt)
                    qk_unit(1, 0, c)
                    qk_unit(0, 0, c)
                pending = attn_pair(0, 0, emit_v_from=KT)
                for pr in range(1, PAIRS):
                    qk_tile(0, pr)
                    qk_tile(1, pr)
                    pending = attn_pair(0, pr, pending=pending)
                # the deferred norm flows across chunk boundaries: chunk
                # qc's last norm runs inside chunk qc+1's first pair, and
                # proj(qc) is emitted after that pair so its matmuls (which
                # wait on the norm chain) never head-block the next chunk's
                # S^T stream in the PE queue
                for qc in range(1, NQC):
                    for pr in range(PAIRS):
                        pending = attn_pair(
                            qc, pr, pending=pending,
                            tail=(qc == NQC - 1 and pr == PAIRS - 1))
                        if pr == 0:
                            proj_chunk(qc - 1)
                pending()
                pending = None
                proj_chunk(NQC - 1)

    nc.finalize()
    return nc


_NC = None


def _get_nc():
    global _NC
    if _NC is None:
        _NC = build_nc()
    return _NC


def _make_in_maps(inputs):
    x = np.asarray(inputs["x"], dtype=np.float32)
    w_qkv = np.asarray(inputs["w_qkv"], dtype=np.float32)
    b_qkv = np.asarray(inputs["b_qkv"], dtype=np.float32)
    w_proj = np.asarray(inputs["w_proj"], dtype=np.float32)

    bf = ml_dtypes.bfloat16
    in_maps = []
    for c in range(8):
        b, g = c // 2, c % 2
        sl = slice(384 * g, 384 * g + 384)
        xT = np.ascontiguousarray(x[b].T)                       # [768, 2048]
        wq = w_qkv[0:768][sl]                                    # [384, 768]
        wk = w_qkv[768:1536][sl]
        wv = w_qkv[1536:2304][sl]
        wqkvT = np.ascontiguousarray(np.concatenate([wq, wk, wv], axis=0).T)
        bq = b_qkv[0:768][sl]
        bk = b_qkv[768:1536][sl]
        bv = b_qkv[1536:2304][sl]
        b_qk = np.ascontiguousarray(
            np.concatenate([bq, bk]).reshape(6, P).T)            # [128, 6]
        wpT = np.ascontiguousarray(w_proj[:, sl].T)
        in_maps.append({
            "onesb": np.ones((P, P), dtype=bf),
            "xT": xT.astype(bf),
            "wqkvT": wqkvT.astype(bf),
            "b_qk": b_qk,
            "b_v": np.ascontiguousarray(bv.reshape(1, 384)).astype(bf),
            "wpT": wpT.astype(bf),
        })
    return in_maps


def _run(inputs, trace=False):
    nc = _get_nc()
    in_maps = _make_in_maps(inputs)
    res = run_bass_kernel_spmd(nc, in_maps, core_ids=list(range(8)), trace=trace)
    b_proj = np.asarray(inputs["b_proj"], dtype=np.float32)
    out = np.empty((4, NQ, CD), dtype=np.float32)
    for b in range(4):
        out[b] = res.results[2 * b]["out"] + res.results[2 * b + 1]["out"] + b_proj
    return out, res


def kernel(**inputs) -> np.ndarray:
    out, _ = _run(inputs, trace=False)
    return out

